# revision 6
# baseline (speedup 1.0000x reference)
"""Trainium2 Bass kernel for multi-head attention (nn_MHA).

Full inputs in, full outputs out.  Internally shards across 8 NeuronCores:
2-way data parallel over batch x 4-way tensor parallel over heads.
Each core computes, for its 2 batches x 4 heads:
  q/k/v projections (fp32r matmuls), scores, softmax (exp on ACT with
  f32 accumulated row sums), the full attention-probability output, the
  attention context, and its partial slice of the output projection.
The host sums the 4 tensor-parallel partial outputs and adds bo.
"""

import sys

for _p in ("/opt/trn_rl_repo", "/root/.axon_site/_ro/trn_rl_repo"):
    if _p not in sys.path:
        sys.path.append(_p)

from contextlib import ExitStack  # noqa: E402

import numpy as np  # noqa: E402

import concourse.bass as bass  # noqa: E402
import concourse.bacc as bacc  # noqa: E402
import concourse.mybir as mybir  # noqa: E402
import concourse.tile as tile  # noqa: E402
from concourse.masks import make_identity  # noqa: E402

F32 = mybir.dt.float32
F32R = mybir.dt.float32r
BF16 = mybir.dt.bfloat16
AF = mybir.ActivationFunctionType

# Full problem dims
EMBED = 1024
HEADS = 16
HEAD_DIM = 64
BATCH = 4
SEQ = 2048
N_CORES = 8
TP = 4  # head groups (tensor parallel)
DP = 2  # batch groups (data parallel)

P = 128  # partitions
SBLK = 512  # moving-dim block (one PSUM bank of f32)


def build_mha_nc(B, S, E, HL, DH, SQ):
    """Build the SPMD Bass program for one core.

    B: local batches, S: seq len, E: embed dim, HL: local heads,
    DH: head dim, SQ: s-tiles per sub-block (attention pipelining unit).
    """
    DL = HL * DH  # local head width (columns of Wq/Wk/Wv, rows of Wo)
    NS = S // P  # s-tiles
    NE = E // P  # e-chunks (contraction tiles for projections)
    NT = S // P  # t-chunks
    NDC = max(DL // P, 1)  # 128-wide chunks of DL
    QT = NS // SQ  # sub-blocks per (b, h)
    assert NS % SQ == 0 and DL % P == 0 or DL == P

    nc = bacc.Bacc()

    xq_d = nc.declare_dram_parameter("xqT", [B, E, S], F32, isOutput=False)
    xk_d = nc.declare_dram_parameter("xkT", [B, E, S], F32, isOutput=False)
    xv_d = nc.declare_dram_parameter("xvT", [B, E, S], F32, isOutput=False)
    wq_d = nc.declare_dram_parameter("Wq", [E, DL], F32, isOutput=False)
    wk_d = nc.declare_dram_parameter("Wk", [E, DL], F32, isOutput=False)
    wv_d = nc.declare_dram_parameter("Wv", [E, DL], F32, isOutput=False)
    bq_d = nc.declare_dram_parameter("bq", [DL], F32, isOutput=False)
    bk_d = nc.declare_dram_parameter("bk", [DL], F32, isOutput=False)
    bv_d = nc.declare_dram_parameter("bv", [DL], F32, isOutput=False)
    wo_d = nc.declare_dram_parameter("Wo", [DL, E], F32, isOutput=False)
    attn_d = nc.declare_dram_parameter("attn", [B, HL, S, S], F32, isOutput=True)
    y_d = nc.declare_dram_parameter("y", [B, S, E], F32, isOutput=True)

    with tile.TileContext(nc) as tc, ExitStack() as ctx:
        const = ctx.enter_context(tc.tile_pool(name="const", bufs=1))
        x_pool = ctx.enter_context(tc.tile_pool(name="xp", bufs=NE))
        qk_pool = ctx.enter_context(tc.tile_pool(name="qkp", bufs=2 * NDC))
        v_pool = ctx.enter_context(tc.tile_pool(name="vp", bufs=1))
        ctx_pool = ctx.enter_context(tc.tile_pool(name="cxp", bufs=NDC))
        exp_pool = ctx.enter_context(tc.tile_pool(name="exp", bufs=SQ + 1))
        attn_pool = ctx.enter_context(tc.tile_pool(name="atp", bufs=SQ + 2))
        expt_pool = ctx.enter_context(tc.tile_pool(name="etp", bufs=3))
        den_pool = ctx.enter_context(tc.tile_pool(name="dnp", bufs=4))
        out_pool = ctx.enter_context(tc.tile_pool(name="otp", bufs=2))
        ps_big = ctx.enter_context(tc.tile_pool(name="psb", bufs=1, space="PSUM"))
        ps_ctx = ctx.enter_context(tc.tile_pool(name="psc", bufs=2, space="PSUM"))
        ps_sm = ctx.enter_context(tc.tile_pool(name="pss", bufs=2, space="PSUM"))

        # ---- constants: weights, biases, identity ----
        wq_sb = const.tile([P, NE, DL], BF16, tag="wq")
        wk_sb = const.tile([P, NE, DL], BF16, tag="wk")
        wv_sb = const.tile([P, NE, DL], BF16, tag="wv")
        nc.gpsimd.dma_start(out=wq_sb, in_=wq_d.rearrange("(c p) d -> p c d", p=P))
        nc.gpsimd.dma_start(out=wk_sb, in_=wk_d.rearrange("(c p) d -> p c d", p=P))
        nc.gpsimd.dma_start(out=wv_sb, in_=wv_d.rearrange("(c p) d -> p c d", p=P))
        wo_sb = const.tile([P, NDC, E], BF16, tag="wo")
        nc.gpsimd.dma_start(out=wo_sb, in_=wo_d.rearrange("(c p) e -> p c e", p=P))
        bq_sb = const.tile([P, NDC], F32, tag="bq")
        bk_sb = const.tile([P, NDC], F32, tag="bk")
        nc.sync.dma_start(out=bq_sb, in_=bq_d.rearrange("(c p) -> p c", p=P))
        nc.sync.dma_start(out=bk_sb, in_=bk_d.rearrange("(c p) -> p c", p=P))
        bv_full = bv_d[:]
        bv_bc = bass.AP(
            tensor=bv_full.tensor, offset=bv_full.offset, ap=[[0, P]] + list(bv_full.ap)
        )
        bv_sb = const.tile([P, DL], F32, tag="bv")
        nc.gpsimd.dma_start(out=bv_sb, in_=bv_bc)
        ident = const.tile([P, P], BF16, tag="ident")
        make_identity(nc, ident)

        for b in range(B):
            # ---- P1: projections ----
            # qT/kT: [DL, S] as NDC tiles of [128, S] bf16 (d on partitions)
            def proj_T(x_d_t, w_sb, b_sb, b_idx):
                xt = [x_pool.tile([P, S], BF16, tag="x", name=f"x{_ec}") for _ec in range(NE)]
                for ec in range(NE):
                    nc.gpsimd.dma_start(out=xt[ec], in_=x_d_t[b_idx, ec * P : (ec + 1) * P, :])
                outs = []
                for dc in range(NDC):
                    ps = ps_big.tile([P, S], F32, tag="big")
                    for ec in range(NE):
                        for sb in range(S // SBLK):
                            nc.tensor.matmul(
                                ps[:, sb * SBLK : (sb + 1) * SBLK],
                                lhsT=w_sb[:, ec, dc * P : (dc + 1) * P],
                                rhs=xt[ec][:, sb * SBLK : (sb + 1) * SBLK],
                                start=(ec == 0),
                                stop=(ec == NE - 1),
                            )
                    t = qk_pool.tile([P, S], BF16, tag="qk")
                    nc.scalar.activation(
                        out=t, in_=ps, func=AF.Identity, bias=b_sb[:, dc : dc + 1]
                    )
                    outs.append(t)
                return outs

            q_tiles = proj_T(xq_d, wq_sb, bq_sb, b)
            k_tiles = proj_T(xk_d, wk_sb, bk_sb, b)

            # v: natural [t, d] layout, [128, NT, DL] bf16
            xvt = [x_pool.tile([P, S], BF16, tag="x", name=f"xv{_ec}") for _ec in range(NE)]
            for ec in range(NE):
                nc.gpsimd.dma_start(out=xvt[ec], in_=xv_d[b, ec * P : (ec + 1) * P, :])
            v_sb = v_pool.tile([P, NT, DL], BF16, tag="v")
            for tc_ in range(NT):
                psv = ps_sm.tile([P, DL], F32, tag="small")
                for ec in range(NE):
                    nc.tensor.matmul(
                        psv,
                        lhsT=xvt[ec][:, tc_ * P : (tc_ + 1) * P],
                        rhs=wv_sb[:, ec, :],
                        start=(ec == 0),
                        stop=(ec == NE - 1),
                    )
                nc.vector.tensor_tensor(
                    out=v_sb[:, tc_, :], in0=psv, in1=bv_sb, op=mybir.AluOpType.add
                )

            # ctxT tiles for this batch: [128, S] bf16 per d-chunk
            ctxt_tiles = [ctx_pool.tile([P, S], BF16, tag="ctxT", name=f"ctxT{_dc}") for _dc in range(NDC)]

            # ---- P2: attention per head ----
            for h in range(HL):
                hdc, hoff = (h * DH) // P, (h * DH) % P
                denom = den_pool.tile([P, NS], F32, tag="den")
                rden = den_pool.tile([P, NS], F32, tag="rden")
                for qq in range(QT):
                    # SubA: scores -> exp -> normalize -> write attn
                    ex_tiles = []
                    for i in range(SQ):
                        st = qq * SQ + i
                        ps = ps_big.tile([P, S], F32, tag="big")
                        for sb in range(S // SBLK):
                            nc.tensor.matmul(
                                ps[:, sb * SBLK : (sb + 1) * SBLK],
                                lhsT=q_tiles[hdc][
                                    hoff : hoff + DH, st * P : (st + 1) * P
                                ],
                                rhs=k_tiles[hdc][hoff : hoff + DH, sb * SBLK : (sb + 1) * SBLK],
                                start=True,
                                stop=True,
                            )
                        ex = exp_pool.tile([P, S], BF16, tag="exp")
                        nc.scalar.activation(
                            out=ex,
                            in_=ps,
                            func=AF.Exp,
                            scale=float(1.0 / np.sqrt(DH)),
                            accum_out=denom[:, st : st + 1],
                        )
                        ex_tiles.append(ex)
                    nc.vector.reciprocal(
                        rden[:, qq * SQ : (qq + 1) * SQ],
                        denom[:, qq * SQ : (qq + 1) * SQ],
                    )
                    at_tiles = []
                    for i in range(SQ):
                        st = qq * SQ + i
                        at = attn_pool.tile([P, S], BF16, tag="attn")
                        nc.vector.tensor_scalar_mul(at, ex_tiles[i], rden[:, st : st + 1])
                        # SWDGE cast bf16 -> f32 on the way to HBM
                        nc.gpsimd.dma_start(
                            out=attn_d[b, h, st * P : (st + 1) * P, :], in_=at
                        )
                        at_tiles.append(at)
                    # SubB: transpose attn tiles, accumulate ctxT
                    cps = ps_ctx.tile([DH, SQ * P], F32, tag="ctx")
                    for c in range(NT):
                        trp = ps_sm.tile([P, SQ * P], BF16, tag="small")
                        for i in range(SQ):
                            nc.tensor.transpose(
                                trp[:, i * P : (i + 1) * P],
                                at_tiles[i][:, c * P : (c + 1) * P],
                                ident,
                            )
                        ept = expt_pool.tile([P, SQ * P], BF16, tag="expT")
                        nc.vector.tensor_copy(out=ept, in_=trp)
                        nc.tensor.matmul(
                            cps,
                            lhsT=v_sb[:, c, h * DH : (h + 1) * DH],
                            rhs=ept,
                            start=(c == 0),
                            stop=(c == NT - 1),
                        )
                    nc.vector.tensor_copy(
                        out=ctxt_tiles[hdc][
                            hoff : hoff + DH, qq * SQ * P : (qq + 1) * SQ * P
                        ],
                        in_=cps,
                    )

            # ---- P3: output projection (partial, summed on host) ----
            for st in range(NS):
                po = ps_big.tile([P, E], F32, tag="big")
                for dc in range(NDC):
                    for eb in range(E // SBLK):
                        nc.tensor.matmul(
                            po[:, eb * SBLK : (eb + 1) * SBLK],
                            lhsT=ctxt_tiles[dc][:, st * P : (st + 1) * P],
                            rhs=wo_sb[:, dc, eb * SBLK : (eb + 1) * SBLK],
                            start=(dc == 0),
                            stop=(dc == NDC - 1),
                        )
                ot = out_pool.tile([P, E], F32, tag="out")
                nc.scalar.copy(out=ot, in_=po)
                nc.sync.dma_start(out=y_d[b, st * P : (st + 1) * P, :], in_=ot)

    nc.finalize()
    return nc


_NC_CACHE = {}


def get_nc(B, S, E, HL, DH, SQ):
    key = (B, S, E, HL, DH, SQ)
    if key not in _NC_CACHE:
        _NC_CACHE[key] = build_mha_nc(B, S, E, HL, DH, SQ)
    return _NC_CACHE[key]


def make_in_maps(query, key, value, Wq, bq, Wk, bk, Wv, bv, Wo):
    """Shard full inputs into per-core input maps (DP over batch, TP over heads)."""
    BL = BATCH // DP
    DL = (HEADS // TP) * HEAD_DIM
    xT = {}
    for name, x in (("xqT", query), ("xkT", key), ("xvT", value)):
        xt = np.ascontiguousarray(np.transpose(x, (0, 2, 1)))  # [B, E, S]
        for g in range(DP):
            xT[(name, g)] = np.ascontiguousarray(xt[g * BL : (g + 1) * BL])
    in_maps = []
    for c in range(N_CORES):
        g, hg = divmod(c, TP)
        cs = slice(hg * DL, (hg + 1) * DL)
        in_maps.append(
            {
                "xqT": xT[("xqT", g)],
                "xkT": xT[("xkT", g)],
                "xvT": xT[("xvT", g)],
                "Wq": np.ascontiguousarray(Wq[:, cs]),
                "Wk": np.ascontiguousarray(Wk[:, cs]),
                "Wv": np.ascontiguousarray(Wv[:, cs]),
                "bq": np.ascontiguousarray(bq[cs]),
                "bk": np.ascontiguousarray(bk[cs]),
                "bv": np.ascontiguousarray(bv[cs]),
                "Wo": np.ascontiguousarray(Wo[cs, :]),
            }
        )
    return in_maps


def gather_outputs(results, bo):
    """Assemble full (out, attn) from per-core results."""
    BL = BATCH // DP
    HLOC = HEADS // TP
    attn = np.empty((BATCH, HEADS, SEQ, SEQ), dtype=np.float32)
    out = np.zeros((BATCH, SEQ, EMBED), dtype=np.float32)
    for c in range(N_CORES):
        g, hg = divmod(c, TP)
        attn[g * BL : (g + 1) * BL, hg * HLOC : (hg + 1) * HLOC] = results[c]["attn"]
        out[g * BL : (g + 1) * BL] += results[c]["y"]
    out += bo.astype(np.float32)
    return out, attn


def run_spmd(in_maps, trace=False, tmpdir=None):
    from concourse.bass_utils import run_bass_kernel_spmd

    nc = get_nc(BATCH // DP, SEQ, EMBED, HEADS // TP, HEAD_DIM, 4)
    return run_bass_kernel_spmd(nc, in_maps, list(range(N_CORES)), trace=trace, tmpdir=tmpdir)


def kernel(query, key, value, Wq, bq, Wk, bk, Wv, bv, Wo, bo):
    query = np.asarray(query, dtype=np.float32)
    key = np.asarray(key, dtype=np.float32)
    value = np.asarray(value, dtype=np.float32)
    in_maps = make_in_maps(
        query,
        key,
        value,
        np.asarray(Wq, np.float32),
        np.asarray(bq, np.float32),
        np.asarray(Wk, np.float32),
        np.asarray(bk, np.float32),
        np.asarray(Wv, np.float32),
        np.asarray(bv, np.float32),
        np.asarray(Wo, np.float32),
    )
    res = run_spmd(in_maps)
    return gather_outputs(res.results, np.asarray(bo, np.float32))
